# revision 32
# baseline (speedup 1.0000x reference)
"""Trainium2 Bass kernel for the masked block-diagonal LSTM net.

Model structure (hardcoded from the problem spec):
  - x_seq [512, 64, 32], recurrent state HID=1088 = 34 blocks x 32.
  - U projections are masked so hidden block j only sees input feature j
    (block 0 additionally sees features 0,1 again via the interaction rows);
    hidden blocks 32,33 receive NO input projection at all.
  - V recurrent matrices are masked block-diagonal -> the 34 blocks evolve
    completely independently through the scan.

Sharding: hidden-block parallel. Cores 0..7 each own 4 input-driven blocks
(128 hidden rows) x the full batch 512. Layout on device is h^T:
[hid on partitions, batch on free dim], so the recurrent matmul, the gate
activations and the state updates all run at full 128-partition width.

Blocks 32,33 are bias-only (no x dependence): their state is identical for
every batch element, so their scalar contribution to the readout (and the
tiny 16-feature static MLP + final sigmoid) is folded into the host-side
unshard step.

Schedule (v4, measured on HW): the per-step period is bound by the serial
per-chunk recurrence loop (MMs -> sigmoid -> cell DVE -> tanh -> h-mult ->
MMs), whose latency scales ~1160 + 9.6*C ns with chunk width C. Three
batch chunks (172/170/170) interleave so each engine stays fed while the
other chunks' chains are in flight; measured ~3.45us/step vs 4.4us for
two 256-wide chunks. The cell gate is pre-scaled x2 so one fused sigmoid
covers all 4 gate banks (tanh(y) = 2*sigmoid(2y) - 1, fixed up by a cheap
4x-mode tensor_scalar on DVE).
"""

import sys

sys.path.insert(0, "/opt/trn_rl_repo")

import numpy as np

B = 512
T = 64
INPUT_SZ = 32
HPF = 32
INTER = [(0, 1), (2, 3)]
NB = INPUT_SZ + len(INTER)  # 34
HID = NB * HPF  # 1088
IN_SZ = INPUT_SZ + 2 * len(INTER)  # 36
F_STAT = 16
N_CORES = 8
BLOCKS_PER_CORE = 4
CORE_HID = BLOCKS_PER_CORE * HPF  # 128
CBS = [172, 170, 170]  # batch-column chunks per step (pipeline granularity)
CHUNKS = len(CBS)
COFF = [0, 172, 342]

_CACHE = {}


def _build_masks():
    um = np.zeros((IN_SZ, HID), np.float32)
    for i in range(INPUT_SZ):
        um[i, i * HPF : (i + 1) * HPF] = 1.0
    for i in range(0, len(INTER), 2):
        um[i + INPUT_SZ, i * HPF : (i + 1) * HPF] = 1.0
        um[i + INPUT_SZ + 1, i * HPF : (i + 1) * HPF] = 1.0
    vm = np.kron(np.eye(NB, dtype=np.float32), np.ones((HPF, HPF), np.float32))
    return um, vm


def _build_program(repeat=1, loop_n=0):
    # loop_n>0 wraps ONE copy of the computation in a hardware For_i loop:
    # program size stays constant, so wall-clock deltas between two loop_n
    # values isolate device execution time from per-call dispatch overhead.
    import concourse.bass as bass
    import concourse.tile as tile
    from concourse import bacc, mybir
    from contextlib import nullcontext

    f32 = mybir.dt.float32
    f16 = mybir.dt.float16
    ACT = mybir.ActivationFunctionType

    nc = bacc.Bacc("TRN2", target_bir_lowering=False, debug=False)

    xf_d = nc.dram_tensor("xf", [5, T * B], f16, kind="ExternalInput").ap()
    wu_d = nc.dram_tensor("wu", [5, 4 * CORE_HID], f16, kind="ExternalInput").ap()
    wv_d = nc.dram_tensor(
        "wv", [CORE_HID, 4 * CORE_HID], f16, kind="ExternalInput"
    ).ap()
    oc_d = nc.dram_tensor("oc", [CORE_HID, 1], f16, kind="ExternalInput").ap()
    part_d = nc.dram_tensor("partial", [1, B], f32, kind="ExternalOutput").ap()

    with tile.TileContext(nc) as tc:
        with (
            tc.tile_pool(name="const", bufs=1) as cpool,
            tc.tile_pool(name="state", bufs=4) as spool,
            tc.tile_pool(name="work", bufs=5) as wpool,
            tc.tile_pool(name="psum", bufs=1, space="PSUM") as ppool,
        ):
            # Preload both activation tables while the weight/x DMAs fly.
            dummy = cpool.tile([1, 2], f32, tag="dummy")
            nc.vector.memset(dummy[:], 0)
            dummy2 = cpool.tile([1, 2], f16, tag="dummy2")
            nc.scalar.activation(dummy2[:, 0:1], dummy[:, 0:1], ACT.Sigmoid)
            nc.scalar.activation(dummy2[:, 1:2], dummy[:, 1:2], ACT.Tanh)

            # xf stored as 4 separate segment tiles, all at partition base 0
            # (4 parallel DMA queues for load bandwidth, but every matmul
            # runs at the default tile position -- no per-step PE tile
            # reconfiguration between U and recurrent matmuls).
            SEG = 4
            SEGC = T * B // SEG  # columns per segment (16 steps)
            wuT = cpool.tile([5, 4 * CORE_HID], f16, tag="wuT")
            nc.sync.dma_start(wuT[:], wu_d[:])
            wu = [wuT[:, g * CORE_HID : (g + 1) * CORE_HID] for g in range(4)]
            wvT = cpool.tile([CORE_HID, 4 * CORE_HID], f16, tag="wvT")
            nc.sync.dma_start(wvT[:], wv_d[:])
            wv = [wvT[:, g * CORE_HID : (g + 1) * CORE_HID] for g in range(4)]
            oc = cpool.tile([CORE_HID, 1], f16, tag="oc")
            nc.sync.dma_start(oc[:], oc_d[:])
            xfs = [
                cpool.tile([5, SEGC], f16, tag=f"xf{s}", name=f"xf{s}")
                for s in range(SEG)
            ]
            # segment 0 split in quarters across Pool/ACT queues so the
            # first steps can start as early as possible
            # first sub-DMA covers just 2 steps (1KB/row) so step 0's
            # U-matmuls can start as early as possible
            Q0 = 2 * B
            nc.gpsimd.dma_start(xfs[0][:, 0:Q0], xf_d[:, 0:Q0])
            Q = (SEGC - Q0) // 3
            nc.scalar.dma_start(xfs[0][:, Q0 : Q0 + Q], xf_d[:, Q0 : Q0 + Q])
            nc.gpsimd.dma_start(
                xfs[0][:, Q0 + Q : Q0 + 2 * Q], xf_d[:, Q0 + Q : Q0 + 2 * Q]
            )
            nc.scalar.dma_start(xfs[0][:, Q0 + 2 * Q :], xf_d[:, Q0 + 2 * Q : SEGC])
            for s in range(1, SEG):
                nc.sync.dma_start(
                    xfs[s][:], xf_d[:, s * SEGC : (s + 1) * SEGC]
                )

            loop_cm = (lambda: tc.For_i(0, loop_n, 1)) if loop_n else None
            for rep in range(repeat):
              with loop_cm() if loop_cm else nullcontext():
                outsb = wpool.tile([1, B], f32, tag="outsb")
                # t=0 needs no h/c (h=c=0: gates from U+b only, c_0 = i*g~).
                hs_t = [None] * CHUNKS
                cs_t = [None] * CHUNKS

                for t in range(T):
                    seg, tl = t // 16, t % 16
                    xcols = lambda ch: xfs[seg][
                        :, tl * B + COFF[ch] : tl * B + COFF[ch] + CBS[ch]
                    ]
                    last = t == 0  # U MM closes the accum group at t=0
                    # psum tiles keep a fixed 256-col gate stride so every
                    # gate slice stays 1KB-aligned (matmul outputs must not
                    # cross a 2KB PSUM bank boundary); only the first
                    # CBS[ch] columns of each gate are used.
                    ps = [
                        ppool.tile(
                            [128, 4, 256], f32, tag=f"ps{ch}", name=f"ps{ch}"
                        )
                        for ch in range(CHUNKS)
                    ]
                    for ch in range(CHUNKS):
                        for g in (0, 1, 2, 3):
                            nc.tensor.matmul(
                                ps[ch][:, g, 0 : CBS[ch]], wu[g], xcols(ch),
                                start=True, stop=last,
                                skip_group_check=True,
                            )
                    if t > 0:
                        for ch in range(CHUNKS):
                            for g in (0, 1, 2, 3):
                                nc.tensor.matmul(
                                    ps[ch][:, g, 0 : CBS[ch]], wv[g], hs_t[ch][:],
                                    start=False, stop=True,
                                    skip_group_check=True,
                                )

                    # gate banks: 0=i, 1=f, 2=o, 3=g(cell, x2-prescaled).
                    # Software-pipelined emission: chunk ch's tanh+h are
                    # emitted right after chunk ch+1's sigmoid+cell, so the
                    # early tanhs fill ACT slots between sigmoids and no
                    # h-mult queues behind a later chunk's cell on the DVE.
                    ifog = [None] * CHUNKS
                    c_new = [None] * CHUNKS

                    def emit_sig_cell(ch):
                        cb = CBS[ch]
                        gt = wpool.tile(
                            [CORE_HID, 4, cb], f16, tag=f"ifog{ch}",
                            name=f"ifog{ch}",
                        )
                        nc.scalar.activation(gt[:], ps[ch][:, :, 0:cb], ACT.Sigmoid)
                        ifog[ch] = gt
                        tg = wpool.tile([CORE_HID, cb], f16, tag=f"tg{ch}",
                                        name=f"tg{ch}")
                        nc.vector.tensor_scalar(
                            tg[:], gt[:, 3], 2.0, -1.0,
                            mybir.AluOpType.mult, mybir.AluOpType.add,
                        )
                        cn = spool.tile([CORE_HID, cb], f16, tag=f"c{ch}",
                                        name=f"c{ch}")
                        if t == 0:
                            nc.vector.tensor_mul(cn[:], gt[:, 0], tg[:])
                        else:
                            t1 = wpool.tile([CORE_HID, cb], f16, tag=f"t1{ch}",
                                            name=f"t1{ch}")
                            nc.vector.tensor_mul(t1[:], gt[:, 1], cs_t[ch][:])
                            t2 = wpool.tile([CORE_HID, cb], f16, tag=f"t2{ch}",
                                            name=f"t2{ch}")
                            nc.vector.tensor_mul(t2[:], gt[:, 0], tg[:])
                            nc.vector.tensor_add(cn[:], t1[:], t2[:])
                        c_new[ch] = cn

                    def emit_tanh_h(ch):
                        cb = CBS[ch]
                        tc_ = wpool.tile([CORE_HID, cb], f16, tag=f"sc{ch}",
                                         name=f"sc{ch}")
                        nc.scalar.activation(tc_[:], c_new[ch][:], ACT.Tanh)
                        h_new = spool.tile([CORE_HID, cb], f16, tag=f"h{ch}",
                                           name=f"h{ch}")
                        nc.vector.tensor_mul(h_new[:], ifog[ch][:, 2], tc_[:])
                        hs_t[ch] = h_new
                        if t == T - 1:
                            # readout partial oc^T @ h emitted inline so the
                            # tail overlaps the other chunks' last steps
                            pr = ppool.tile(
                                [128, 4, 256], f32, tag=f"ps{ch}",
                                name=f"pr{ch}",
                            )
                            nc.tensor.matmul(
                                pr[0:1, 0, 0:cb], oc[:], h_new[:],
                                start=True, stop=True, skip_group_check=True,
                            )
                            nc.vector.tensor_copy(
                                outsb[:, COFF[ch] : COFF[ch] + cb],
                                pr[0:1, 0, 0:cb],
                            )

                    for ch in range(CHUNKS):
                        emit_sig_cell(ch)
                    for ch in range(CHUNKS):
                        emit_tanh_h(ch)
                    cs_t = list(c_new)

                nc.sync.dma_start(part_d[:], outsb[:])

    nc.compile()
    return nc


def _pack_inputs(inputs):
    um, vm = _build_masks()
    gates = [
        (inputs["U_i"], inputs["V_i"], inputs["b_i"]),
        (inputs["U_f"], inputs["V_f"], inputs["b_f"]),
        (inputs["U_o"], inputs["V_o"], inputs["b_o"]),
        (inputs["U_c"], inputs["V_c"], inputs["b_c"]),
    ]
    Up = [np.asarray(U, np.float32) * um for U, _, _ in gates]
    Vp = [np.asarray(V, np.float32) * vm for _, V, _ in gates]
    bs = [np.asarray(b, np.float32) for _, _, b in gates]
    x_seq = np.asarray(inputs["x_seq"], np.float32)
    out_coef = np.asarray(inputs["out_coef"], np.float32)

    in_maps = []
    for core in range(N_CORES):
        feats = list(range(4 * core, 4 * core + 4))
        hs = slice(CORE_HID * core, CORE_HID * (core + 1))
        xf = np.ones((5, T * B), np.float32)
        # column index = t*B + b
        xf[0:4] = x_seq[:, :, feats].transpose(2, 1, 0).reshape(4, T * B)
        wu = np.zeros((4, 5, CORE_HID), np.float32)
        wv = np.zeros((4, CORE_HID, CORE_HID), np.float32)
        for g in range(4):
            wu[g, 0:4] = Up[g][feats, hs]
            if core == 0:
                # interaction rows 32,33 multiply x0,x1 -> fold into rows 0,1
                wu[g, 0] += Up[g][32, hs]
                wu[g, 1] += Up[g][33, hs]
            wu[g, 4] = bs[g][hs]
            wv[g] = Vp[g][hs, hs]
        # cell gate (idx 3) pre-scaled x2: tanh(y) = 2*sigmoid(2y) - 1
        wu[3] *= 2.0
        wv[3] *= 2.0
        in_maps.append(
            {
                "xf": xf.astype(np.float16),
                "wu": np.ascontiguousarray(
                    wu.transpose(1, 0, 2).reshape(5, 4 * CORE_HID)
                ).astype(np.float16),
                "wv": np.ascontiguousarray(
                    wv.transpose(1, 0, 2).reshape(CORE_HID, 4 * CORE_HID)
                ).astype(np.float16),
                "oc": np.ascontiguousarray(out_coef[hs]).astype(np.float16),
            }
        )
    return in_maps, Vp, bs, out_coef


def _host_tail(inputs, partials, Vp, bs, out_coef):
    """Bias-only blocks 32,33 (batch-independent scalar) + static MLP +
    final sigmoid. All exact model math, done during unshard."""
    aux = slice(32 * HPF, HID)  # hid 1024:1088
    h = np.zeros(2 * HPF, np.float32)
    cst = np.zeros(2 * HPF, np.float32)
    Va = [V[aux, aux] for V in Vp]
    ba = [b[aux] for b in bs]

    def sig(x):
        return 1.0 / (1.0 + np.exp(-x))

    for _ in range(T):
        i_t = sig(ba[0] + h @ Va[0])
        f_t = sig(ba[1] + h @ Va[1])
        o_t = sig(ba[2] + h @ Va[2])
        g_t = np.tanh(ba[3] + h @ Va[3])
        cst = f_t * cst + i_t * g_t
        h = o_t * np.tanh(cst)
    s_aux = float(h @ out_coef[aux, 0])

    x_stat = np.asarray(inputs["x_stat"], np.float32)
    W1 = np.asarray(inputs["W1"], np.float32)
    b1 = np.asarray(inputs["b1"], np.float32)
    W2 = np.asarray(inputs["W2"], np.float32)
    b2 = np.asarray(inputs["b2"], np.float32)
    hid = np.maximum(x_stat[:, :, None] * W1[None] + b1[None], 0.0)
    mlp = sig(np.einsum("bfk,fk->bf", hid, W2) + b2)
    mlp_part = mlp @ out_coef[HID:, 0]

    z = partials.sum(axis=0) + s_aux + mlp_part + float(np.asarray(inputs["out_bias"])[0])
    return sig(z).astype(np.float32).reshape(B, 1)


def kernel(**inputs):
    from concourse.bass_utils import run_bass_kernel_spmd

    if "nc" not in _CACHE:
        _CACHE["nc"] = _build_program()
    nc = _CACHE["nc"]

    in_maps, Vp, bs, out_coef = _pack_inputs(inputs)
    res = run_bass_kernel_spmd(nc, in_maps, core_ids=list(range(N_CORES)))
    partials = np.stack([res.results[c]["partial"][0] for c in range(N_CORES)])
    return _host_tail(inputs, partials, Vp, bs, out_coef)


# revision 34
# speedup vs baseline: 1.0158x; 1.0158x over previous
"""Trainium2 Bass kernel for the masked block-diagonal LSTM net.

Model structure (hardcoded from the problem spec):
  - x_seq [512, 64, 32], recurrent state HID=1088 = 34 blocks x 32.
  - U projections are masked so hidden block j only sees input feature j
    (block 0 additionally sees features 0,1 again via the interaction rows);
    hidden blocks 32,33 receive NO input projection at all.
  - V recurrent matrices are masked block-diagonal -> the 34 blocks evolve
    completely independently through the scan.

Sharding: hidden-block parallel. Cores 0..7 each own 4 input-driven blocks
(128 hidden rows) x the full batch 512. Layout on device is h^T:
[hid on partitions, batch on free dim], so the recurrent matmul, the gate
activations and the state updates all run at full 128-partition width.

Blocks 32,33 are bias-only (no x dependence): their state is identical for
every batch element, so their scalar contribution to the readout (and the
tiny 16-feature static MLP + final sigmoid) is folded into the host-side
unshard step.

Schedule (v4, measured on HW): the per-step period is bound by the serial
per-chunk recurrence loop (MMs -> sigmoid -> cell DVE -> tanh -> h-mult ->
MMs), whose latency scales ~1160 + 9.6*C ns with chunk width C. Three
batch chunks (172/170/170) interleave so each engine stays fed while the
other chunks' chains are in flight; measured ~3.45us/step vs 4.4us for
two 256-wide chunks. The cell gate is pre-scaled x2 so one fused sigmoid
covers all 4 gate banks (tanh(y) = 2*sigmoid(2y) - 1, fixed up by a cheap
4x-mode tensor_scalar on DVE).
"""

import sys

sys.path.insert(0, "/opt/trn_rl_repo")

import numpy as np

B = 512
T = 64
INPUT_SZ = 32
HPF = 32
INTER = [(0, 1), (2, 3)]
NB = INPUT_SZ + len(INTER)  # 34
HID = NB * HPF  # 1088
IN_SZ = INPUT_SZ + 2 * len(INTER)  # 36
F_STAT = 16
N_CORES = 8
BLOCKS_PER_CORE = 4
CORE_HID = BLOCKS_PER_CORE * HPF  # 128
CBS = [172, 170, 170]  # batch-column chunks per step (pipeline granularity)
CHUNKS = len(CBS)
COFF = [0, 172, 342]

_CACHE = {}


def _build_masks():
    um = np.zeros((IN_SZ, HID), np.float32)
    for i in range(INPUT_SZ):
        um[i, i * HPF : (i + 1) * HPF] = 1.0
    for i in range(0, len(INTER), 2):
        um[i + INPUT_SZ, i * HPF : (i + 1) * HPF] = 1.0
        um[i + INPUT_SZ + 1, i * HPF : (i + 1) * HPF] = 1.0
    vm = np.kron(np.eye(NB, dtype=np.float32), np.ones((HPF, HPF), np.float32))
    return um, vm


def _build_program(repeat=1, loop_n=0):
    # loop_n>0 wraps ONE copy of the computation in a hardware For_i loop:
    # program size stays constant, so wall-clock deltas between two loop_n
    # values isolate device execution time from per-call dispatch overhead.
    import concourse.bass as bass
    import concourse.tile as tile
    from concourse import bacc, mybir
    from contextlib import nullcontext

    f32 = mybir.dt.float32
    f16 = mybir.dt.float16
    ACT = mybir.ActivationFunctionType

    nc = bacc.Bacc("TRN2", target_bir_lowering=False, debug=False)

    xf_d = nc.dram_tensor("xf", [5, T * B], f16, kind="ExternalInput").ap()
    wu_d = nc.dram_tensor("wu", [5, 4 * CORE_HID], f16, kind="ExternalInput").ap()
    wv_d = nc.dram_tensor(
        "wv", [CORE_HID, 4 * CORE_HID], f16, kind="ExternalInput"
    ).ap()
    oc_d = nc.dram_tensor("oc", [CORE_HID, 1], f16, kind="ExternalInput").ap()
    part_d = nc.dram_tensor("partial", [1, B], f32, kind="ExternalOutput").ap()

    with tile.TileContext(nc) as tc:
        with (
            tc.tile_pool(name="const", bufs=1) as cpool,
            tc.tile_pool(name="state", bufs=3) as spool,
            tc.tile_pool(name="work", bufs=5) as wpool,
            tc.tile_pool(name="psum", bufs=1, space="PSUM") as ppool,
        ):
            # Preload both activation tables while the weight/x DMAs fly.
            dummy = cpool.tile([1, 2], f32, tag="dummy")
            nc.vector.memset(dummy[:], 0)
            dummy2 = cpool.tile([1, 2], f16, tag="dummy2")
            nc.scalar.activation(dummy2[:, 0:1], dummy[:, 0:1], ACT.Sigmoid)
            nc.scalar.activation(dummy2[:, 1:2], dummy[:, 1:2], ACT.Tanh)

            # xf stored as 4 separate segment tiles, all at partition base 0
            # (4 parallel DMA queues for load bandwidth, but every matmul
            # runs at the default tile position -- no per-step PE tile
            # reconfiguration between U and recurrent matmuls).
            SEG = 4
            SEGC = T * B // SEG  # columns per segment (16 steps)
            wuT = cpool.tile([5, 4 * CORE_HID], f16, tag="wuT")
            nc.sync.dma_start(wuT[:], wu_d[:])
            wu = [wuT[:, g * CORE_HID : (g + 1) * CORE_HID] for g in range(4)]
            wvT = cpool.tile([CORE_HID, 4 * CORE_HID], f16, tag="wvT")
            nc.sync.dma_start(wvT[:], wv_d[:])
            wv = [wvT[:, g * CORE_HID : (g + 1) * CORE_HID] for g in range(4)]
            oc = cpool.tile([CORE_HID, 1], f16, tag="oc")
            nc.sync.dma_start(oc[:], oc_d[:])
            xfs = [
                cpool.tile([5, SEGC], f16, tag=f"xf{s}", name=f"xf{s}")
                for s in range(SEG)
            ]
            # segment 0 split in quarters across Pool/ACT queues so the
            # first steps can start as early as possible
            # first sub-DMA covers just 2 steps (1KB/row) so step 0's
            # U-matmuls can start as early as possible
            Q0 = 2 * B
            nc.gpsimd.dma_start(xfs[0][:, 0:Q0], xf_d[:, 0:Q0])
            Q = (SEGC - Q0) // 3
            nc.scalar.dma_start(xfs[0][:, Q0 : Q0 + Q], xf_d[:, Q0 : Q0 + Q])
            nc.gpsimd.dma_start(
                xfs[0][:, Q0 + Q : Q0 + 2 * Q], xf_d[:, Q0 + Q : Q0 + 2 * Q]
            )
            nc.scalar.dma_start(xfs[0][:, Q0 + 2 * Q :], xf_d[:, Q0 + 2 * Q : SEGC])
            for s in range(1, SEG):
                nc.sync.dma_start(
                    xfs[s][:], xf_d[:, s * SEGC : (s + 1) * SEGC]
                )

            loop_cm = (lambda: tc.For_i(0, loop_n, 1)) if loop_n else None
            for rep in range(repeat):
              with loop_cm() if loop_cm else nullcontext():
                outsb = wpool.tile([1, B], f32, tag="outsb")
                # t=0 needs no h/c (h=c=0: gates from U+b only, c_0 = i*g~).
                hs_t = [None] * CHUNKS
                cs_t = [None] * CHUNKS

                for t in range(T):
                    seg, tl = t // 16, t % 16
                    xcols = lambda ch: xfs[seg][
                        :, tl * B + COFF[ch] : tl * B + COFF[ch] + CBS[ch]
                    ]
                    last = t == 0  # U MM closes the accum group at t=0
                    # psum tiles keep a fixed 256-col gate stride so every
                    # gate slice stays 1KB-aligned (matmul outputs must not
                    # cross a 2KB PSUM bank boundary); only the first
                    # CBS[ch] columns of each gate are used.
                    ps = [
                        ppool.tile(
                            [128, 4, 256], f32, tag=f"ps{ch}", name=f"ps{ch}"
                        )
                        for ch in range(CHUNKS)
                    ]
                    for ch in range(CHUNKS):
                        for g in (0, 1, 2, 3):
                            nc.tensor.matmul(
                                ps[ch][:, g, 0 : CBS[ch]], wu[g], xcols(ch),
                                start=True, stop=last,
                                skip_group_check=True,
                            )
                    if t > 0:
                        for ch in range(CHUNKS):
                            for g in (0, 1, 2, 3):
                                nc.tensor.matmul(
                                    ps[ch][:, g, 0 : CBS[ch]], wv[g], hs_t[ch][:],
                                    start=False, stop=True,
                                    skip_group_check=True,
                                )

                    # gate banks: 0=i, 1=f, 2=o, 3=g(cell, x2-prescaled).
                    # Software-pipelined emission: chunk ch's tanh+h are
                    # emitted right after chunk ch+1's sigmoid+cell, so the
                    # early tanhs fill ACT slots between sigmoids and no
                    # h-mult queues behind a later chunk's cell on the DVE.
                    ifog = [None] * CHUNKS
                    c_new = [None] * CHUNKS

                    def emit_sig_cell(ch):
                        cb = CBS[ch]
                        gt = wpool.tile(
                            [CORE_HID, 4, cb], f16, tag=f"ifog{ch}",
                            name=f"ifog{ch}",
                        )
                        nc.scalar.activation(gt[:], ps[ch][:, :, 0:cb], ACT.Sigmoid)
                        ifog[ch] = gt
                        tg = wpool.tile([CORE_HID, cb], f16, tag=f"tg{ch}",
                                        name=f"tg{ch}")
                        nc.vector.tensor_scalar(
                            tg[:], gt[:, 3], 2.0, -1.0,
                            mybir.AluOpType.mult, mybir.AluOpType.add,
                        )
                        cn = spool.tile([CORE_HID, cb], f16, tag=f"c{ch}",
                                        name=f"c{ch}")
                        if t == 0:
                            nc.vector.tensor_mul(cn[:], gt[:, 0], tg[:])
                        else:
                            t1 = wpool.tile([CORE_HID, cb], f16, tag=f"t1{ch}",
                                            name=f"t1{ch}")
                            nc.vector.tensor_mul(t1[:], gt[:, 1], cs_t[ch][:])
                            t2 = wpool.tile([CORE_HID, cb], f16, tag=f"t2{ch}",
                                            name=f"t2{ch}")
                            nc.vector.tensor_mul(t2[:], gt[:, 0], tg[:])
                            nc.vector.tensor_add(cn[:], t1[:], t2[:])
                        c_new[ch] = cn

                    def emit_tanh_h(ch):
                        cb = CBS[ch]
                        tc_ = wpool.tile([CORE_HID, cb], f16, tag=f"sc{ch}",
                                         name=f"sc{ch}")
                        nc.scalar.activation(tc_[:], c_new[ch][:], ACT.Tanh)
                        h_new = spool.tile([CORE_HID, cb], f16, tag=f"h{ch}",
                                           name=f"h{ch}")
                        nc.vector.tensor_mul(h_new[:], ifog[ch][:, 2], tc_[:])
                        hs_t[ch] = h_new
                        if t == T - 1:
                            # readout partial oc^T @ h emitted inline so the
                            # tail overlaps the other chunks' last steps
                            pr = ppool.tile(
                                [128, 4, 256], f32, tag=f"ps{ch}",
                                name=f"pr{ch}",
                            )
                            nc.tensor.matmul(
                                pr[0:1, 0, 0:cb], oc[:], h_new[:],
                                start=True, stop=True, skip_group_check=True,
                            )
                            nc.vector.tensor_copy(
                                outsb[:, COFF[ch] : COFF[ch] + cb],
                                pr[0:1, 0, 0:cb],
                            )

                    for ch in range(CHUNKS):
                        emit_sig_cell(ch)
                    for ch in range(CHUNKS):
                        emit_tanh_h(ch)
                    cs_t = list(c_new)

                nc.sync.dma_start(part_d[:], outsb[:])

    nc.compile()
    return nc


def _pack_inputs(inputs):
    um, vm = _build_masks()
    gates = [
        (inputs["U_i"], inputs["V_i"], inputs["b_i"]),
        (inputs["U_f"], inputs["V_f"], inputs["b_f"]),
        (inputs["U_o"], inputs["V_o"], inputs["b_o"]),
        (inputs["U_c"], inputs["V_c"], inputs["b_c"]),
    ]
    Up = [np.asarray(U, np.float32) * um for U, _, _ in gates]
    Vp = [np.asarray(V, np.float32) * vm for _, V, _ in gates]
    bs = [np.asarray(b, np.float32) for _, _, b in gates]
    x_seq = np.asarray(inputs["x_seq"], np.float32)
    out_coef = np.asarray(inputs["out_coef"], np.float32)

    in_maps = []
    for core in range(N_CORES):
        feats = list(range(4 * core, 4 * core + 4))
        hs = slice(CORE_HID * core, CORE_HID * (core + 1))
        xf = np.ones((5, T * B), np.float32)
        # column index = t*B + b
        xf[0:4] = x_seq[:, :, feats].transpose(2, 1, 0).reshape(4, T * B)
        wu = np.zeros((4, 5, CORE_HID), np.float32)
        wv = np.zeros((4, CORE_HID, CORE_HID), np.float32)
        for g in range(4):
            wu[g, 0:4] = Up[g][feats, hs]
            if core == 0:
                # interaction rows 32,33 multiply x0,x1 -> fold into rows 0,1
                wu[g, 0] += Up[g][32, hs]
                wu[g, 1] += Up[g][33, hs]
            wu[g, 4] = bs[g][hs]
            wv[g] = Vp[g][hs, hs]
        # cell gate (idx 3) pre-scaled x2: tanh(y) = 2*sigmoid(2y) - 1
        wu[3] *= 2.0
        wv[3] *= 2.0
        in_maps.append(
            {
                "xf": xf.astype(np.float16),
                "wu": np.ascontiguousarray(
                    wu.transpose(1, 0, 2).reshape(5, 4 * CORE_HID)
                ).astype(np.float16),
                "wv": np.ascontiguousarray(
                    wv.transpose(1, 0, 2).reshape(CORE_HID, 4 * CORE_HID)
                ).astype(np.float16),
                "oc": np.ascontiguousarray(out_coef[hs]).astype(np.float16),
            }
        )
    return in_maps, Vp, bs, out_coef


def _host_tail(inputs, partials, Vp, bs, out_coef):
    """Bias-only blocks 32,33 (batch-independent scalar) + static MLP +
    final sigmoid. All exact model math, done during unshard."""
    aux = slice(32 * HPF, HID)  # hid 1024:1088
    h = np.zeros(2 * HPF, np.float32)
    cst = np.zeros(2 * HPF, np.float32)
    Va = [V[aux, aux] for V in Vp]
    ba = [b[aux] for b in bs]

    def sig(x):
        return 1.0 / (1.0 + np.exp(-x))

    for _ in range(T):
        i_t = sig(ba[0] + h @ Va[0])
        f_t = sig(ba[1] + h @ Va[1])
        o_t = sig(ba[2] + h @ Va[2])
        g_t = np.tanh(ba[3] + h @ Va[3])
        cst = f_t * cst + i_t * g_t
        h = o_t * np.tanh(cst)
    s_aux = float(h @ out_coef[aux, 0])

    x_stat = np.asarray(inputs["x_stat"], np.float32)
    W1 = np.asarray(inputs["W1"], np.float32)
    b1 = np.asarray(inputs["b1"], np.float32)
    W2 = np.asarray(inputs["W2"], np.float32)
    b2 = np.asarray(inputs["b2"], np.float32)
    hid = np.maximum(x_stat[:, :, None] * W1[None] + b1[None], 0.0)
    mlp = sig(np.einsum("bfk,fk->bf", hid, W2) + b2)
    mlp_part = mlp @ out_coef[HID:, 0]

    z = partials.sum(axis=0) + s_aux + mlp_part + float(np.asarray(inputs["out_bias"])[0])
    return sig(z).astype(np.float32).reshape(B, 1)


def kernel(**inputs):
    from concourse.bass_utils import run_bass_kernel_spmd

    if "nc" not in _CACHE:
        _CACHE["nc"] = _build_program()
    nc = _CACHE["nc"]

    in_maps, Vp, bs, out_coef = _pack_inputs(inputs)
    res = run_bass_kernel_spmd(nc, in_maps, core_ids=list(range(N_CORES)))
    partials = np.stack([res.results[c]["partial"][0] for c in range(N_CORES)])
    return _host_tail(inputs, partials, Vp, bs, out_coef)
